# revision 1
# baseline (speedup 1.0000x reference)
"""Trainium2 Bass kernel for nn_Joiner (RNN-T joiner: dense_mlp).

Reference computation (per batch n):
  enc = encoder_out @ W_enc.T + b_enc           (T=200, J=512)
  dec = decoder_out @ W_dec.T + b_dec           (U=50,  J=512)
  act = tanh(enc[:,None,:] + dec[None,:,:])     (T, U, J)
  out = act @ W_out.T + b_out                   (T, U, V=500)

Sharding: data-parallel over batch N=8 -> one batch element per NeuronCore.
Per-core layout: J on partitions (4 chunks of 128) for enc/dec/act;
positions pos = t*U+u flattened t-major so PSUM results [pos, V] DMA out
as fully contiguous blocks. Output matmul runs in float32r (full PE rate
at N=500 moving dim).
"""

import numpy as np

N, T, U = 8, 200, 50
E = D = J = 512
V = 500
P = 128
JC = J // P  # 4 j-chunks
T_BLOCKS = [(0, 64), (64, 64), (128, 64), (192, 8)]  # t0, tb (tb*U % 128 == 0 except tail)

_CACHE = {}


def _split_multi_waits(nc, mybir):
    """Walrus's PE (S3_LW) codegen accepts at most one sync-wait per
    instruction. Tile can emit several. Move every wait of a multi-wait
    instruction onto single-wait NOPs inserted just before it (same engine,
    in-order execution makes this equivalent)."""
    n = 0
    for fn in nc.m.functions:
        for blk in fn.blocks:
            new_insts = []
            for inst in blk.instructions:
                si = inst.sync_info
                if si is not None and len(si.on_wait) > 1:
                    for w in si.on_wait:
                        nop = mybir.InstNoOp(
                            name=f"waitnop-{n}",
                            ins=[],
                            outs=[],
                            sync_info=mybir.SyncInfo(on_wait=[w], on_update=[]),
                            bass_nofuse=True,
                        )
                        n += 1
                        nop.engine = inst.engine
                        new_insts.append(nop)
                    inst.sync_info = mybir.SyncInfo(
                        on_wait=[], on_update=si.on_update
                    )
                new_insts.append(inst)
            blk.instructions[:] = new_insts
    return n


def _build_nc():
    import concourse.bass as bass
    import concourse.tile as tile
    from concourse import mybir

    f32 = mybir.dt.float32
    f32r = mybir.dt.float32r
    AF = mybir.ActivationFunctionType
    ALU = mybir.AluOpType

    nc = bass.Bass("TRN2", target_bir_lowering=False, debug=False, num_devices=8)

    enc_t_d = nc.dram_tensor("enc_t", [E, T], f32, kind="ExternalInput").ap()
    dec_t_d = nc.dram_tensor("dec_t", [D, U], f32, kind="ExternalInput").ap()
    w_encT_d = nc.dram_tensor("w_encT", [E, J], f32, kind="ExternalInput").ap()
    w_decT_d = nc.dram_tensor("w_decT", [D, J], f32, kind="ExternalInput").ap()
    w_outT_d = nc.dram_tensor("w_outT", [J, V], f32, kind="ExternalInput").ap()
    bsum_d = nc.dram_tensor("bsum", [P, JC], f32, kind="ExternalInput").ap()
    b_out_d = nc.dram_tensor("b_out_bc", [P, V], f32, kind="ExternalInput").ap()
    out_d = nc.dram_tensor("out", [T * U, V], f32, kind="ExternalOutput").ap()

    with tile.TileContext(nc) as tc:
        with (
            tc.tile_pool(name="consts", bufs=1) as consts,
            tc.tile_pool(name="act", bufs=2) as act_pool,
            tc.tile_pool(name="stage", bufs=3) as stage_pool,
            tc.tile_pool(name="psum", bufs=2, space="PSUM") as psum_pool,
        ):
            # ---- load inputs ----
            enc_raw = consts.tile([P, JC, T], f32, tag="enc_raw")
            nc.sync.dma_start(enc_raw[:], enc_t_d.rearrange("(c p) t -> p c t", p=P))
            w_enc_sb = consts.tile([P, JC, J], f32, tag="w_enc")
            nc.sync.dma_start(w_enc_sb[:], w_encT_d.rearrange("(c p) j -> p c j", p=P))
            dec_raw = consts.tile([P, JC, U], f32, tag="dec_raw")
            nc.sync.dma_start(dec_raw[:], dec_t_d.rearrange("(c p) u -> p c u", p=P))
            w_dec_sb = consts.tile([P, JC, J], f32, tag="w_dec")
            nc.sync.dma_start(w_dec_sb[:], w_decT_d.rearrange("(c p) j -> p c j", p=P))
            bsum_sb = consts.tile([P, JC], f32, tag="bsum")
            nc.sync.dma_start(bsum_sb[:], bsum_d)
            w_out_sb = consts.tile([P, JC, V], f32, tag="w_out")
            nc.sync.dma_start(w_out_sb[:], w_outT_d.rearrange("(c p) v -> p c v", p=P))
            b_out_sb = consts.tile([P, V], f32, tag="b_out")
            nc.sync.dma_start(b_out_sb[:], b_out_d)
            # fp32r matmul operands must be produced by a rounding op (BIR
            # verifier); cast the output weights once.
            w_out_r = consts.tile([P, JC, V], f32r, tag="w_out_r")
            nc.vector.tensor_copy(out=w_out_r[:], in_=w_out_sb[:])
            # K=1 bias matmul operands: ones row and b_out row, both fp32r.
            ones_f32 = consts.tile([1, P], f32, tag="ones_f32")
            nc.vector.memset(ones_f32[:], 1.0)
            ones_r = consts.tile([1, P], f32r, tag="ones_r")
            nc.vector.tensor_copy(out=ones_r[:], in_=ones_f32[:])
            b_out_row_r = consts.tile([1, V], f32r, tag="b_out_row_r")
            nc.vector.tensor_copy(out=b_out_row_r[:], in_=b_out_sb[0:1, :])

            # ---- projections: enc_sb[j, t], dec_sb[j, u] (J on partitions, 4 chunks) ----
            enc_sb = consts.tile([P, JC, T], f32, tag="enc_sb")
            dec_sb = consts.tile([P, JC, U], f32, tag="dec_sb")
            for jb in range(JC):
                ps = psum_pool.tile([P, 4, 512], f32, tag="psumg")
                pe = ps[:, 0, :T]
                for ec in range(JC):
                    nc.tensor.matmul(
                        pe,
                        lhsT=w_enc_sb[:, ec, jb * P:(jb + 1) * P],
                        rhs=enc_raw[:, ec, :],
                        start=(ec == 0),
                        stop=(ec == JC - 1),
                    )
                nc.scalar.copy(out=enc_sb[:, jb, :], in_=pe)
                pd = ps[:, 1, :U]
                for ec in range(JC):
                    nc.tensor.matmul(
                        pd,
                        lhsT=w_dec_sb[:, ec, jb * P:(jb + 1) * P],
                        rhs=dec_raw[:, ec, :],
                        start=(ec == 0),
                        stop=(ec == JC - 1),
                    )
                # dec_sb = dec_proj + (b_enc + b_dec)  (fold both biases here)
                nc.scalar.add(out=dec_sb[:, jb, :], in_=pd, add=bsum_sb[:, jb:jb + 1])

            # ---- main loop over T blocks ----
            for (t0, tb) in T_BLOCKS:
                npos = tb * U
                act = act_pool.tile([P, JC, npos], f32r, tag="act")
                for jb in range(JC):
                    act3 = act[:, jb, :].rearrange("p (t u) -> p t u", u=U)
                    enc_bc = enc_sb[:, jb, t0:t0 + tb][:, :, None].to_broadcast([P, tb, U])
                    dec_bc = dec_sb[:, jb, None, :].to_broadcast([P, tb, U])
                    nc.vector.tensor_tensor(out=act3, in0=enc_bc, in1=dec_bc, op=ALU.add)
                    nc.scalar.activation(out=act[:, jb, :], in_=act[:, jb, :], func=AF.Tanh)

                # pos tiles of 128, grouped 4 per PSUM allocation (4 banks)
                tiles = []
                p0 = 0
                while p0 < npos:
                    sz = min(P, npos - p0)
                    tiles.append((p0, sz))
                    p0 += sz
                base = t0 * U
                for gstart in range(0, len(tiles), 4):
                    group = tiles[gstart:gstart + 4]
                    ng = len(group)
                    uniform = all(sz == P for (_, sz) in group)
                    # Alternate bias+copy route to balance DVE vs PE/ACT load:
                    # route_b folds b_out via an extra K=1 fp32r matmul and
                    # copies PSUM->SBUF on ScalarE; route_a adds b_out on DVE.
                    route_b = uniform and (gstart // 4) % 3 == 0
                    psum_g = psum_pool.tile([P, 4, 512], f32, tag="psumg")
                    for i, (ls, sz) in enumerate(group):
                        for jb in range(JC):
                            nc.tensor.matmul(
                                psum_g[:sz, i, :V],
                                lhsT=act[:, jb, ls:ls + sz],
                                rhs=w_out_r[:, jb, :],
                                start=(jb == 0),
                                stop=(jb == JC - 1) and not route_b,
                            )
                        if route_b:
                            nc.tensor.matmul(
                                psum_g[:sz, i, :V],
                                lhsT=ones_r[:, :sz],
                                rhs=b_out_row_r[:],
                                start=False,
                                stop=True,
                            )
                    stage = stage_pool.tile([P, 4, V], f32, tag="stage")
                    if uniform and route_b:
                        nc.scalar.copy(out=stage[:, :ng, :], in_=psum_g[:, :ng, :V])
                        dst = out_d[base + group[0][0]: base + group[0][0] + ng * P, :]
                        nc.sync.dma_start(
                            dst.rearrange("(g p) v -> p g v", p=P), stage[:, :ng, :]
                        )
                    elif uniform:
                        nc.vector.tensor_tensor(
                            out=stage[:, :ng, :],
                            in0=psum_g[:, :ng, :V],
                            in1=b_out_sb[:, None, :].to_broadcast([P, ng, V]),
                            op=ALU.add,
                        )
                        dst = out_d[base + group[0][0]: base + group[0][0] + ng * P, :]
                        nc.sync.dma_start(
                            dst.rearrange("(g p) v -> p g v", p=P), stage[:, :ng, :]
                        )
                    else:
                        for i, (ls, sz) in enumerate(group):
                            nc.vector.tensor_tensor(
                                out=stage[:sz, i, :],
                                in0=psum_g[:sz, i, :V],
                                in1=b_out_sb[:sz, :],
                                op=ALU.add,
                            )
                            nc.sync.dma_start(
                                out_d[base + ls: base + ls + sz, :], stage[:sz, i, :]
                            )
    _split_multi_waits(nc, mybir)
    return nc


def _prep_inputs(encoder_out, decoder_out, W_enc, b_enc, W_dec, b_dec, W_out, b_out):
    encoder_out = np.ascontiguousarray(encoder_out, dtype=np.float32)
    decoder_out = np.ascontiguousarray(decoder_out, dtype=np.float32)
    w_encT = np.ascontiguousarray(np.asarray(W_enc, np.float32).T)
    w_decT = np.ascontiguousarray(np.asarray(W_dec, np.float32).T)
    w_outT = np.ascontiguousarray(np.asarray(W_out, np.float32).T)
    bsum = np.ascontiguousarray(
        (np.asarray(b_enc, np.float32) + np.asarray(b_dec, np.float32)).reshape(JC, P).T
    )
    b_out_bc = np.ascontiguousarray(np.tile(np.asarray(b_out, np.float32)[None, :], (P, 1)))
    in_maps = []
    for n in range(N):
        in_maps.append({
            "enc_t": np.ascontiguousarray(encoder_out[n].T),
            "dec_t": np.ascontiguousarray(decoder_out[n].T),
            "w_encT": w_encT,
            "w_decT": w_decT,
            "w_outT": w_outT,
            "bsum": bsum,
            "b_out_bc": b_out_bc,
        })
    return in_maps


def get_nc():
    if "nc" not in _CACHE:
        _CACHE["nc"] = _build_nc()
    return _CACHE["nc"]


def run_on_hw(in_maps, trace=False):
    from concourse.bass_utils import run_bass_kernel_spmd

    nc = get_nc()
    return run_bass_kernel_spmd(nc, in_maps, core_ids=list(range(N)), trace=trace)


def kernel(encoder_out, decoder_out, W_enc, b_enc, W_dec, b_dec, W_out, b_out):
    in_maps = _prep_inputs(
        encoder_out, decoder_out, W_enc, b_enc, W_dec, b_dec, W_out, b_out
    )
    res = run_on_hw(in_maps)
    out = np.stack([res.results[i]["out"] for i in range(N)], axis=0)
    return out.reshape(N, T, U, V)

